# revision 23
# baseline (speedup 1.0000x reference)
"""Distributed Trainium2 (Bass/Tile) kernel for the AdaMEOW GNN loss.

Front half: target-node dim N row-sharded across 8 cores; neighbor dim M
sharded for the neighbor MLPs.  Aggregation is node-major so the two
per-view ReduceScatter payloads (fp8) write with contiguous descriptors;
RS0 issues while view-1 still computes.  One fp8 AllGather shares y1
(first GCN matmul output, [N,64] x 5 graphs).

Back half is REPLICATED: every core holds the full fp8 adjacency
matrices (prefetched on idle DMA rings) and computes the full-graph GCN
layers with fp8 DoubleRow matmuls, so the attention logits e, beta, and
the full zc come out locally -- no further collectives.  A cheap local
shard-z path supplies the per-core anchor rows (zf, diag, A).  InfoNCE
pair weights w[i,j]=sigmoid(sum_h tanh(A+B)*m2+b2) use a DMA-broadcast
of B with tanh on ACT and two DVE FMA chains.

Small constants ride in two packed tensors; counts are input metadata
(1/clamp(cnt,1) host-side).  Activation-table switches are grouped
(Exp -> Tanh -> Exp -> Tanh -> Ln) via tanh/exp identities.
"""

import os

import ml_dtypes
import numpy as np

import concourse.bass as bass
import concourse.mybir as mybir
import concourse.tile as tile
from concourse import bacc
from concourse.bass_utils import run_bass_kernel_spmd

FP = mybir.dt.float32
BF = mybir.dt.bfloat16
F8 = mybir.dt.float8e4
NPBF = ml_dtypes.bfloat16
NPF8 = ml_dtypes.float8_e4m3fn
AF = mybir.ActivationFunctionType
ALU = mybir.AluOpType
DR = mybir.MatmulPerfMode.DoubleRow

N, M, D0, D1, H, E = 1024, 4096, 1024, 512, 512, 64
C = 8
NL = N // C
ML = M // C
P = 128
HK = H // P      # 4
D0K = D0 // P    # 8
MLK = ML // P    # 4
NB = N // P      # 8
TAU = 0.5
RG = [list(range(C))]

# packed f32 consts [128, CW]: fc0b(0:4) b2half(4) b1bc(5:21) rec2(21:23)
# gcnb1(23) gcnb2(24) attb(25) attv(26) projb(27) projb2(28) m2f(29:45)
CW = 45
# packed bf16 consts [128, CB]: ident(0:128) gcnw1(128:384) gcnw2(384:448)
# attw(448:512) projw(512:576) mlp1w(576:592) fc1b(592:1104) fc2b(1104:1616)
CB = 1616


def _build():
    nc = bacc.Bacc("TRN2", num_devices=C)

    def din(name, shape, dt=BF):
        return nc.declare_dram_parameter(name, list(shape), dt, isOutput=False)

    fm0T = din("fm0T", (D0, 2 * NL))
    feat1T = din("feat1T", (D1, ML))
    feat2T = din("feat2T", (D1, ML))
    nei0T = din("nei0T", (ML, N), F8)
    nei1T = din("nei1T", (ML, N), F8)
    adjF = [din(f"adjF{i}", (N, N), F8) for i in range(4)]   # a0 m0 a1 m1 ^T
    adjS = [din(f"adjS{i}", (N, NL), F8) for i in range(4)]  # shard cols
    fc0_w = din("fc0_w", (D0, H))
    fc1_w = din("fc1_w", (D1, H))
    fc2_w = din("fc2_w", (D1, H))
    agg0_w = din("agg0_w", (H, H))
    agg1_w = din("agg1_w", (H, H))
    gcnw2_8 = din("gcnw2_8", (E, E), F8)
    cf32 = din("cf32", (P, CW), FP)
    cbf = din("cbf", (P, CB), BF)

    out_ext = nc.declare_dram_parameter("out", [1, 1], FP, isOutput=True)

    rs_in = [nc.dram_tensor(f"rs{v}_in", [NB, NL, H], F8) for v in range(2)]
    rs_out = [nc.dram_tensor(f"rs{v}_out", [NL, H], F8) for v in range(2)]
    ag1_in = nc.dram_tensor("ag1_in", [5 * P, E], F8)
    ag1_out = nc.dram_tensor("ag1_out", [C * 5 * P, E], F8, addr_space="Shared")
    bt_d = nc.dram_tensor("bt_d", [1, 16 * N], F8)

    with tile.TileContext(nc) as tc:
        with (
            tc.tile_pool(name="pers", bufs=1) as pers,
            tc.tile_pool(name="wkW", bufs=2) as wkW,
            tc.tile_pool(name="wkT", bufs=3) as wkT,
            tc.tile_pool(name="wkS", bufs=3) as wkS,
            tc.tile_pool(name="psW", bufs=4, space="PSUM") as psW,
            tc.tile_pool(name="psS", bufs=4, space="PSUM") as psS,
        ):
            def mk(pool, shape, name, dt=FP):
                return pool.tile(list(shape), dt, tag=name, name=name)

            def wsm(shape, name="tsm"):
                return wkS.tile(list(shape), FP, tag=name, name=name)

            def ld(pool, dram, shape, name, pat=None, eng=None, **kw):
                t = mk(pool, shape, name, dt=dram.dtype)
                src = dram[:] if pat is None else dram[:].rearrange(pat, **kw)
                (eng or nc.sync).dma_start(t[:], src)
                return t

            with tc.tile_pool(name="s1", bufs=1) as s1:
                # -------- wave-1 loads ---------------------------------
                cf_sb = ld(pers, cf32, (P, CW), "cf32", eng=nc.scalar)
                cb_sb = ld(pers, cbf, (P, CB), "cbf", eng=nc.scalar)
                feat1T_sb = ld(s1, feat1T, (P, MLK, ML), "feat1T",
                               "(o p) f -> p o f", p=P, eng=nc.sync)
                fc1w_sb = ld(s1, fc1_w, (P, MLK, H), "fc1w",
                             "(o p) f -> p o f", p=P, eng=nc.scalar)
                nei0T_sb = ld(s1, nei0T, (P, MLK, N), "nei0T",
                              "(o p) f -> p o f", p=P, eng=nc.sync)
                feat2T_sb = ld(s1, feat2T, (P, MLK, ML), "feat2T",
                               "(o p) f -> p o f", p=P, eng=nc.scalar)
                fc2w_sb = ld(s1, fc2_w, (P, MLK, H), "fc2w",
                             "(o p) f -> p o f", p=P, eng=nc.scalar)
                nei1T_sb = ld(s1, nei1T, (P, MLK, N), "nei1T",
                              "(o p) f -> p o f", p=P, eng=nc.scalar)
                fm0T_sb = ld(s1, fm0T, (P, D0K, 2 * NL), "fm0T",
                             "(o p) f -> p o f", p=P, eng=nc.sync)
                fc0w_sb = ld(s1, fc0_w, (P, D0K, H), "fc0w",
                             "(o p) f -> p o f", p=P, eng=nc.sync)
                agg0w_sb = ld(s1, agg0_w, (P, HK, H), "agg0w",
                              "(o p) f -> p o f", p=P, eng=nc.sync)
                agg1w_sb = ld(s1, agg1_w, (P, HK, H), "agg1w",
                              "(o p) f -> p o f", p=P, eng=nc.scalar)
                gw2_8 = ld(pers, gcnw2_8, (E, E), "gw2_8", eng=nc.scalar)

                # -------- packed-const views ---------------------------
                fc0b = cf_sb[:, 0:4]
                b2half = cf_sb[:, 4:5]
                b1bc = cf_sb[:, 5:21]
                rec = cf_sb[:, 21:23]
                gcnb1 = cf_sb[0:E, 23:24]
                gcnb2 = cf_sb[0:E, 24:25]
                attb = cf_sb[0:E, 25:26]
                attv = cf_sb[0:E, 26:27]
                projb = cf_sb[0:E, 27:28]
                projb2 = cf_sb[0:E, 28:29]
                m2f = cf_sb[:, 29:45]
                ident = cb_sb[:, 0:P]
                gcnw1 = cb_sb[:, 128:384].rearrange("p (o e) -> p o e", o=HK)
                attw = cb_sb[0:E, 448:512]
                projw = cb_sb[0:E, 512:576]
                mlp1w = cb_sb[0:E, 576:592]
                fc1b_bc = cb_sb[:, 592:1104]
                fc2b_bc = cb_sb[:, 1104:1616]

                ones_col = mk(pers, (P, 1), "ones_col", BF)
                nc.vector.memset(ones_col[:], 1.0)
                ones_row = mk(pers, (1, P), "ones_row", BF)
                nc.vector.memset(ones_row[:], 1.0)
                onesf_col = mk(pers, (P, 1), "onesf_col", FP)
                nc.vector.memset(onesf_col[:], 1.0)

                htmT_sb = mk(s1, (P, HK, 2 * NL), "htmT", BF)
                zcT_sb = mk(pers, (E, NL), "zcT", BF)
                zfT_sb = mk(pers, (E, NL), "zfT", BF)
                zfineT = mk(pers, (E, NL), "zfineT", BF)

                def elu_wide(x_ap, out_ap, fdim):
                    # r = max(x,0)-1 ; out = exp(min(x,0)) + r
                    r = wkW.tile([P, fdim], FP, tag="elu_r", name="elu_r")
                    m = wkW.tile([P, fdim], FP, tag="elu_m", name="elu_m")
                    nc.vector.tensor_scalar(
                        out=r[:], in0=x_ap, scalar1=0.0, scalar2=-1.0,
                        op0=ALU.max, op1=ALU.add)
                    nc.vector.tensor_scalar_min(m[:], x_ap, 0.0)
                    nc.scalar.activation(m[:], m[:], AF.Exp)
                    nc.gpsimd.tensor_add(out_ap, m[:], r[:])

                # -------- stage 1: h_nei MLPs + node-major agg ---------
                hnei8 = [mk(s1, (P, MLK, H), "hnei0", F8),
                         mk(s1, (P, MLK, H), "hnei1", F8)]
                stage = [mk(s1, (P, NB, H), "stage0", F8),
                         mk(s1, (P, NB, H), "stage1", F8)]
                neis = [nei0T_sb, nei1T_sb]
                weng = [nc.sync, nc.scalar]

                for v, (fT, fw, fbc) in enumerate(
                    [(feat1T_sb, fc1w_sb, fc1b_bc),
                     (feat2T_sb, fc2w_sb, fc2b_bc)]
                ):
                    for mc in range(MLK):
                        ps = psW.tile([P, H], FP, tag="psW", name="ps_hnei")
                        for k in range(MLK):
                            nc.tensor.matmul(
                                ps[:], fT[:, k, mc * P:(mc + 1) * P],
                                fw[:, k, :],
                                start=(k == 0), stop=(k == MLK - 1))
                        xb = wkW.tile([P, H], FP, tag="xb", name="xb")
                        nc.vector.tensor_add(xb[:], ps[:], fbc)
                        elu_wide(xb[:], hnei8[v][:, mc, :], H)

                    for b in range(NB):
                        ps = psW.tile([P, H], FP, tag="psW", name="ps_agg")
                        for k2 in range(2):
                            nc.tensor.matmul(
                                ps[:],
                                neis[v][:, 2 * k2:2 * k2 + 2,
                                        b * P:(b + 1) * P],
                                hnei8[v][:, 2 * k2:2 * k2 + 2, :],
                                start=(k2 == 0), stop=(k2 == 1),
                                perf_mode=DR)
                        nc.vector.tensor_copy(stage[v][:, b, :], ps[:])
                    weng[v].dma_start(
                        rs_in[v][:].rearrange("b p f -> p b f"), stage[v][:])
                    nc.gpsimd.collective_compute(
                        "ReduceScatter", ALU.add, replica_groups=RG,
                        ins=[rs_in[v][:].opt()], outs=[rs_out[v][:].opt()])

                # full adjacencies (fp8, needed only post-AG1); issued
                # after stage-1 loads so they ride the RS idle window
                adjF_sb = [ld(pers, adjF[i], (P, NB, N), f"adjF{i}",
                              "(o p) f -> p o f", p=P, eng=nc.gpsimd)
                           for i in range(4)]
                adjS_sb = [ld(pers, adjS[i], (P, NB, NL), f"adjS{i}",
                              "(o p) f -> p o f", p=P, eng=nc.gpsimd)
                           for i in range(4)]

                # -------- h_tar | h_mask (RS shadow) -------------------
                for hc in range(HK):
                    ps = psS.tile([P, 2 * NL], FP, tag="psS", name="ps_htm")
                    for k in range(D0K):
                        nc.tensor.matmul(
                            ps[:], fc0w_sb[:, k, hc * P:(hc + 1) * P],
                            fm0T_sb[:, k, :],
                            start=(k == 0), stop=(k == D0K - 1))
                    xb2 = wkW.tile([P, 2 * NL], FP, tag="xb2", name="xb2")
                    nc.vector.tensor_scalar_add(
                        xb2[:], ps[:], fc0b[:, hc:hc + 1])
                    elu_wide(xb2[:], htmT_sb[:, hc, :], 2 * NL)

                # full + shard mean adjacency (coarse graph)
                meanF_sb = mk(pers, (P, NB, N), "meanF", F8)
                nc.vector.tensor_add(meanF_sb[:], adjF_sb[0][:], adjF_sb[2][:])
                meanS_sb = mk(pers, (P, NB, NL), "meanS", F8)
                nc.vector.tensor_add(meanS_sb[:], adjS_sb[0][:], adjS_sb[2][:])
                adjsF = [meanF_sb, adjF_sb[0], adjF_sb[1],
                         adjF_sb[2], adjF_sb[3]]
                adjsS = [meanS_sb, adjS_sb[0], adjS_sb[1],
                         adjS_sb[2], adjS_sb[3]]

                st5a = mk(s1, (P, 5, E), "st5a", F8)
                ps = psS.tile([P, E], FP, tag="psS", name="ps_y1c")
                for k in range(HK):
                    nc.tensor.matmul(
                        ps[:], htmT_sb[:, k, 0:NL], gcnw1[:, k, :],
                        start=(k == 0), stop=(k == HK - 1))
                nc.vector.tensor_scalar_mul(st5a[:, 0, :], ps[:], 0.5)

                # -------- views from the two RS results ----------------
                aggT = [mk(s1, (P, HK, NL), "aggT0", BF),
                        mk(s1, (P, HK, NL), "aggT1", BF)]
                xs_sb = [mk(s1, (P, HK, NL), f"x_{t}", BF)
                         for t in ("v0", "m0", "v1", "m1")]
                for v in range(2):
                    agg_sb = mk(s1, (NL, H), f"aggnm{v}", F8)
                    weng[v].dma_start(agg_sb[:], rs_out[v][:])
                    aggs = mk(s1, (NL, H), f"aggs{v}", BF)
                    nc.vector.tensor_scalar_mul(
                        aggs[:], agg_sb[:], rec[:, v:v + 1])
                    for k in range(HK):
                        pst = psW.tile([P, P], BF, tag="psW", name="ps_tr")
                        nc.tensor.transpose(
                            pst[:], aggs[:, k * P:(k + 1) * P], ident)
                        nc.vector.tensor_copy(aggT[v][:, k, :], pst[:])
                    aggw = [agg0w_sb, agg1w_sb][v]
                    xpre = [wkW.tile([P, HK, NL], FP, tag="xpre0",
                                     name="xpre0"),
                            wkW.tile([P, HK, NL], FP, tag="xpre1",
                                     name="xpre1")]
                    for hc in range(HK):
                        ps_t = psS.tile([P, P], FP, tag="psS", name="ps_t")
                        for k in range(HK):
                            nc.tensor.matmul(
                                ps_t[:], aggw[:, k, hc * P:(hc + 1) * P],
                                aggT[v][:, k, :],
                                start=(k == 0), stop=(k == HK - 1))
                        for ti, toff in ((0, 0), (1, NL)):
                            nc.vector.scalar_tensor_tensor(
                                out=xpre[ti][:, hc, :], in0=ps_t[:],
                                scalar=1.0,
                                in1=htmT_sb[:, hc, toff:toff + NL],
                                op0=ALU.bypass, op1=ALU.add)
                    for ti in range(2):
                        elu_wide(xpre[ti][:], xs_sb[2 * v + ti][:], HK * NL)

                for g in range(4):
                    ps = psS.tile([P, E], FP, tag="psS", name="ps_y1")
                    for k in range(HK):
                        nc.tensor.matmul(
                            ps[:], xs_sb[g][:, k, :], gcnw1[:, k, :],
                            start=(k == 0), stop=(k == HK - 1))
                    nc.vector.tensor_copy(st5a[:, 1 + g, :], ps[:])
                nc.sync.dma_start(
                    ag1_in[:].rearrange("(g p) e -> p g e", p=P), st5a[:])
                nc.gpsimd.collective_compute(
                    "AllGather", ALU.bypass, replica_groups=RG,
                    ins=[ag1_in[:].opt()], outs=[ag1_out[:].opt()])

            # ================= late: replicated GCN ===================
            with tc.tile_pool(name="late", bufs=1) as late:
                y1_sb = mk(late, (P, 5 * C, E), "y1", F8)
                y1src = ag1_out[:].rearrange("(o p) e -> p o e", p=P)
                nc.sync.dma_start(y1_sb[:, 0:20, :], y1src[:, 0:20, :])
                nc.scalar.dma_start(y1_sb[:, 20:40, :], y1src[:, 20:40, :])
                y1v = y1_sb[:].rearrange("p (s g) e -> p g s e", g=5)

                # layer 1 (full graph): h = relu(adj @ y1 + b1)
                h8T = mk(late, (E, 5, N), "h8T", F8)
                for gi in range(5):
                    for jh in range(2):
                        ps = psW.tile([E, 512], FP, tag="psW", name="ps_h")
                        for k2 in range(HK):
                            nc.tensor.matmul(
                                ps[:], y1v[:, gi, 2 * k2:2 * k2 + 2, :],
                                adjsF[gi][:, 2 * k2:2 * k2 + 2,
                                          jh * 512:(jh + 1) * 512],
                                start=(k2 == 0), stop=(k2 == HK - 1),
                                perf_mode=DR)
                        nc.vector.tensor_scalar(
                            out=h8T[:, gi, jh * 512:(jh + 1) * 512],
                            in0=ps[:], scalar1=gcnb1, scalar2=0.0,
                            op0=ALU.add, op1=ALU.max)

                # y2 = h @ w2 (node-major blocks, fp8)
                y2v = mk(late, (P, 5, NB, E), "y2v", F8)
                for gi in range(5):
                    ps2 = psW.tile([P, NB * E], FP, tag="psW", name="ps_y2")
                    for s in range(NB):
                        nc.tensor.matmul(
                            ps2[:, s * E:(s + 1) * E],
                            h8T[:, gi, s * P:(s + 1) * P], gw2_8[:])
                    if gi == 0:
                        nc.vector.tensor_scalar_mul(
                            y2v[:, gi, :, :], ps2[:], 0.5)
                    else:
                        nc.vector.tensor_copy(y2v[:, gi, :, :], ps2[:])

                # layer 2 full: z = adj @ y2 + b2  (for e and zc)
                z0F = mk(late, (E, N), "z0F", BF)
                z4F = mk(late, (E, 4, N), "z4F", F8)
                def zfull_of(gi):
                    for jh in range(2):
                        ps = psW.tile([E, 512], FP, tag="psW", name="ps_z")
                        for k2 in range(HK):
                            nc.tensor.matmul(
                                ps[:], y2v[:, gi, 2 * k2:2 * k2 + 2, :],
                                adjsF[gi][:, 2 * k2:2 * k2 + 2,
                                          jh * 512:(jh + 1) * 512],
                                start=(k2 == 0), stop=(k2 == HK - 1),
                                perf_mode=DR)
                        zdst = (z0F[:, jh * 512:(jh + 1) * 512] if gi == 0
                                else z4F[:, gi - 1, jh * 512:(jh + 1) * 512])
                        nc.vector.tensor_scalar_add(zdst, ps[:], gcnb2)

                zfull_of(0)

                # layer 2 shard: local anchor columns
                zTL = mk(late, (E, 5, NL), "zTL", BF)
                for gi in range(5):
                    ps = psS.tile([E, NL], FP, tag="psS", name="ps_zl")
                    for k2 in range(HK):
                        nc.tensor.matmul(
                            ps[:], y2v[:, gi, 2 * k2:2 * k2 + 2, :],
                            adjsS[gi][:, 2 * k2:2 * k2 + 2, :],
                            start=(k2 == 0), stop=(k2 == HK - 1),
                            perf_mode=DR)
                    nc.vector.tensor_scalar_add(zTL[:, gi, :], ps[:], gcnb2)

                def rnorm_chunk(sq_chunk):
                    # bf16 [1,512] reciprocal column norms of one chunk
                    psn = psS.tile([1, 512], FP, tag="psS", name="ps_nc")
                    nc.tensor.matmul(psn[:], ones_col[0:E, :], sq_chunk)
                    nrc = wkS.tile([1, 512], FP, tag="nrc", name="nrc")
                    nc.scalar.activation(nrc[:], psn[:], AF.Sqrt)
                    rcc = wkS.tile([1, 512], FP, tag="rcc", name="rcc")
                    nc.vector.reciprocal(rcc[:], nrc[:])
                    rccb = wkS.tile([1, 512], BF, tag="rccb", name="rccb")
                    nc.vector.tensor_copy(rccb[:], rcc[:])
                    psb = psW.tile([E, 512], FP, tag="psW", name="ps_bcn")
                    nc.tensor.matmul(psb[:], ones_row[:, 0:E], rccb[:])
                    return psb

                # ---- zc full: tanh(z0 @ projw + b), l2norm ----------
                zcF = mk(late, (E, N), "zcF", BF)
                tcz = mk(late, (E, N), "tcz", BF)
                for jh in range(2):
                    psp = psW.tile([E, 512], FP, tag="psW", name="ps_pc")
                    nc.tensor.matmul(psp[:], projw,
                                     z0F[:, jh * 512:(jh + 1) * 512])
                    nc.scalar.activation(
                        tcz[:, jh * 512:(jh + 1) * 512], psp[:], AF.Tanh,
                        bias=projb)
                sqc = mk(late, (E, N), "sqc", F8)
                nc.gpsimd.tensor_mul(sqc[:], tcz[:], tcz[:])
                for jh in range(2):
                    psb = rnorm_chunk(sqc[:, jh * 512:(jh + 1) * 512])
                    nc.vector.tensor_mul(
                        zcF[:, jh * 512:(jh + 1) * 512],
                        tcz[:, jh * 512:(jh + 1) * 512], psb[:])

                # BT = mlp1w^T @ zc_full -> DRAM -> partition broadcast
                BT_sb = mk(late, (16, N), "BT", F8)
                for jh in range(2):
                    pst = psS.tile([16, 512], FP, tag="psS", name="ps_BT")
                    nc.tensor.matmul(pst[:], mlp1w,
                                     zcF[:, jh * 512:(jh + 1) * 512])
                    nc.vector.tensor_copy(BT_sb[:, jh * 512:(jh + 1) * 512],
                                          pst[:])
                nc.scalar.dma_start(
                    bt_d[:].rearrange("one (h f) -> h (one f)", h=16),
                    BT_sb[:])
                bbc = mk(late, (P, 16, N), "bbc", F8)
                bbeng = [nc.sync, nc.scalar, nc.sync, nc.scalar]
                for q in range(4):
                    bbeng[q].dma_start(
                        bbc[:, 4 * q:4 * (q + 1), :],
                        bt_d[:, 4 * q * N:4 * (q + 1) * N].to_broadcast(
                            (P, 4 * N)))

                for gi in range(1, 5):
                    zfull_of(gi)

                # ---- l2norm of 4 fine views (full) + attention ------
                hsF = mk(late, (E, 4, N), "hsF", BF)
                z4 = z4F[:]
                sq4 = mk(late, (E, 4 * N), "sq4", F8)
                nc.gpsimd.tensor_mul(sq4[:], z4, z4)
                eacc = wsm((E, 8), "eacc")
                z4flat = z4.rearrange("e v n -> e (v n)")
                hsflat = hsF[:].rearrange("e v n -> e (v n)")
                for c in range(8):
                    psb = rnorm_chunk(sq4[:, c * 512:(c + 1) * 512])
                    hs_c = hsflat[:, c * 512:(c + 1) * 512]
                    nc.vector.tensor_mul(
                        hs_c, z4flat[:, c * 512:(c + 1) * 512], psb[:])
                    # attention partial on this chunk (Tanh + accum)
                    psa = psW.tile([E, 512], FP, tag="psW", name="ps_att")
                    nc.tensor.matmul(psa[:], attw, hs_c)
                    junk = wkS.tile([E, 512], BF, tag="attj", name="attj")
                    nc.scalar.activation(
                        junk[:], psa[:], AF.Tanh, bias=attb,
                        accum_out=eacc[:, c:c + 1])

                ev = wsm((E, 4), "ev")
                eview = eacc[:].rearrange("e (v j) -> e v j", j=2)
                nc.vector.tensor_add(ev[:], eview[:, :, 0], eview[:, :, 1])
                rv = wsm((E, 4), "rv")
                nc.vector.tensor_scalar(
                    out=rv[:], in0=ev[:], scalar1=attv, scalar2=0.0,
                    op0=ALU.mult, op1=ALU.add)
                rvb = wkS.tile([E, 4], BF, tag="rvb", name="rvb")
                nc.vector.tensor_copy(rvb[:], rv[:])
                pse = psS.tile([1, 4], FP, tag="psS", name="ps_e")
                nc.tensor.matmul(pse[:], ones_col[0:E, :], rvb[:])

                # beta softmax: exp via tanh identity (scale folds 1/N)
                te = wsm((1, 4), "te")
                nc.scalar.activation(te[:], pse[:], AF.Tanh, scale=0.5 / N)
                nm = wsm((1, 4), "nm")
                nc.vector.tensor_scalar_add(nm[:], te[:], 1.0)
                dn = wsm((1, 4), "dn")
                nc.vector.tensor_scalar(
                    out=dn[:], in0=te[:], scalar1=-1.0, scalar2=1.0,
                    op0=ALU.mult, op1=ALU.add)
                rcd = wsm((1, 4), "rcd")
                nc.vector.reciprocal(rcd[:], dn[:])
                ee = wsm((1, 4), "ee")
                nc.vector.tensor_mul(ee[:], nm[:], rcd[:])
                se = wsm((1, 1), "se")
                nc.vector.reduce_sum(se[:], ee[:], axis=mybir.AxisListType.X)
                nc.vector.reciprocal(se[:], se[:])
                beta_row = wkS.tile([1, 4], BF, tag="beta", name="beta")
                nc.vector.tensor_scalar_mul(beta_row[:], ee[:], se[:])
                psbb = psS.tile([P, 4], FP, tag="psS", name="ps_beta")
                nc.tensor.matmul(psbb[:], ones_row[:], beta_row[:])
                beta_bc = wsm((P, 4), "beta_bc")
                nc.vector.tensor_copy(beta_bc[:], psbb[:])

                # ---- local shard: hs, zfine, zf, zc_loc -------------
                def colnorm_loc(src_ap, out_ap):
                    sqb = wkS.tile([E, NL], BF, tag="sqb", name="sqb")
                    nc.vector.tensor_mul(sqb[:], src_ap, src_ap)
                    psn = psS.tile([1, NL], FP, tag="psS", name="ps_norm")
                    nc.tensor.matmul(psn[:], ones_col[0:E, :], sqb[:])
                    nr = wsm((1, NL), "nr")
                    nc.scalar.activation(nr[:], psn[:], AF.Sqrt)
                    rc = wsm((1, NL), "rc")
                    nc.vector.reciprocal(rc[:], nr[:])
                    rcb = wkS.tile([1, NL], BF, tag="rcb", name="rcb")
                    nc.vector.tensor_copy(rcb[:], rc[:])
                    psb2 = psS.tile([P, NL], FP, tag="psS", name="ps_nbc")
                    nc.tensor.matmul(psb2[:], ones_row[:], rcb[:])
                    nc.vector.tensor_mul(out_ap, src_ap, psb2[0:E, :])

                # zc_loc (Tanh table) for the diagonal
                psl0 = psS.tile([E, NL], FP, tag="psS", name="ps_pl")
                nc.tensor.matmul(psl0[:], projw, zTL[:, 0, :])
                tcl = wsm((E, NL), "tcl")
                nc.scalar.activation(tcl[:], psl0[:], AF.Tanh, bias=projb)
                colnorm_loc(tcl[:], zcT_sb[:])

                # hs local (batched l2norm over [E, 4*NL])
                hsL = mk(late, (E, 4, NL), "hsL", BF)
                z4l = zTL[:, 1:5, :]
                sql = mk(late, (E, 4 * NL), "sql", BF)
                nc.gpsimd.tensor_mul(sql[:], z4l, z4l)
                psn4 = psS.tile([1, 4 * NL], FP, tag="psS", name="ps_n4")
                nc.tensor.matmul(psn4[:], ones_col[0:E, :], sql[:])
                nr4 = wkS.tile([1, 4 * NL], FP, tag="nr4", name="nr4")
                nc.scalar.activation(nr4[:], psn4[:], AF.Sqrt)
                rc4 = wkS.tile([1, 4 * NL], FP, tag="rc4", name="rc4")
                nc.vector.reciprocal(rc4[:], nr4[:])
                rc4b = wkS.tile([1, 4 * NL], BF, tag="rc4b", name="rc4b")
                nc.vector.tensor_copy(rc4b[:], rc4[:])
                psb4 = psS.tile([P, 4 * NL], FP, tag="psS", name="ps_nb4")
                nc.tensor.matmul(psb4[:], ones_row[:], rc4b[:])
                nc.vector.tensor_mul(hsL[:], z4l, psb4[0:E, :])

                nc.vector.tensor_scalar_mul(
                    zfineT[:], hsL[:, 0, :], beta_bc[0:E, 0:1])
                for v in range(1, 4):
                    nc.vector.scalar_tensor_tensor(
                        out=zfineT[:], in0=hsL[:, v, :],
                        scalar=beta_bc[0:E, v:v + 1], in1=zfineT[:],
                        op0=ALU.mult, op1=ALU.add)

                # zf proj: tanh(x) = 1 - 2/(exp(2x+2b)+1)  (Exp table)
                pspf = psS.tile([E, NL], FP, tag="psS", name="ps_pf")
                nc.tensor.matmul(pspf[:], projw, zfineT[:])
                e2 = wsm((E, NL), "e2t")
                nc.scalar.activation(e2[:], pspf[:], AF.Exp, scale=2.0,
                                     bias=projb2)
                dnf = wsm((E, NL), "dnf")
                nc.vector.tensor_scalar_add(dnf[:], e2[:], 1.0)
                rcf = wsm((E, NL), "rcf")
                nc.vector.reciprocal(rcf[:], dnf[:])
                tff = wsm((E, NL), "tff")
                nc.vector.tensor_scalar(
                    out=tff[:], in0=rcf[:], scalar1=-2.0, scalar2=1.0,
                    op0=ALU.mult, op1=ALU.add)
                colnorm_loc(tff[:], zfT_sb[:])

                # ---- InfoNCE --------------------------------------
                prod = wsm((E, NL), "prod")
                nc.gpsimd.tensor_mul(prod[:], zfT_sb[:], zcT_sb[:])
                psd = psS.tile([NL, 1], FP, tag="psS", name="ps_diag")
                nc.tensor.matmul(psd[:], prod[:], onesf_col[0:E, :])
                diag_sb = wsm((NL, 1), "diag")
                nc.vector.tensor_scalar_mul(diag_sb[:], psd[:], 1.0 / TAU)

                dots_sb = mk(late, (P, N), "dots", BF)
                for jh in range(2):
                    psq = psW.tile([P, 512], FP, tag="psW", name="ps_log")
                    nc.tensor.matmul(psq[:], zfT_sb[:],
                                     zcF[:, jh * 512:(jh + 1) * 512])
                    nc.scalar.activation(
                        dots_sb[:, jh * 512:(jh + 1) * 512], psq[:], AF.Exp,
                        scale=1.0 / TAU)

                psa2 = psS.tile([NL, 16], FP, tag="psS", name="ps_A")
                nc.tensor.matmul(psa2[:], zfT_sb[:], mlp1w)
                A_sb = mk(late, (NL, 16), "A")
                nc.vector.tensor_add(A_sb[:], psa2[:], b1bc[0:NL, :])

                # pair-weight accumulation: two DVE chains
                acc0 = mk(late, (P, N), "acc0", BF)
                acc1 = mk(late, (P, N), "acc1", BF)
                for hh in range(16):
                    th = wkT.tile([P, N], BF, tag="th", name="th")
                    nc.scalar.activation(
                        th[:], bbc[:, hh, :], AF.Tanh,
                        bias=A_sb[:, hh:hh + 1])
                    dst = [acc0, acc1][hh % 2][:]
                    if hh < 2:
                        nc.vector.tensor_scalar_mul(
                            dst, th[:], m2f[:, hh:hh + 1])
                    else:
                        nc.vector.scalar_tensor_tensor(
                            out=dst, in0=th[:], scalar=m2f[:, hh:hh + 1],
                            in1=dst, op0=ALU.mult, op1=ALU.add)
                nc.gpsimd.tensor_add(acc0[:], acc0[:], acc1[:])

                # sigmoid(x+b2) = 0.5*tanh(0.5x+0.5b2)+0.5
                th2 = wkT.tile([P, N], BF, tag="th", name="th2")
                nc.scalar.activation(th2[:], acc0[:], AF.Tanh, scale=0.5,
                                     bias=b2half)
                w_sb = mk(late, (P, N), "w", BF)
                nc.vector.tensor_scalar(
                    out=w_sb[:], in0=th2[:], scalar1=0.5, scalar2=0.5,
                    op0=ALU.mult, op1=ALU.add)

                denom_sb = wsm((P, 1), "denom")
                junk2 = wkT.tile([P, N], BF, tag="th", name="junk2")
                nc.vector.scalar_tensor_tensor(
                    out=junk2[:], in0=dots_sb[:], scalar=1.0, in1=w_sb[:],
                    op0=ALU.bypass, op1=ALU.mult, accum_out=denom_sb[:])

                lnd = wsm((P, 1), "lnd")
                nc.scalar.activation(lnd[:], denom_sb[:], AF.Ln)
                diff = wsm((P, 1), "diff")
                nc.vector.tensor_sub(diff[:], lnd[:], diag_sb[:])
                psf = psS.tile([1, 1], FP, tag="psS", name="ps_loss")
                nc.tensor.matmul(psf[:], diff[:], onesf_col[:])
                res = wsm((1, 1), "res")
                nc.vector.tensor_copy(res[:], psf[:])
                nc.sync.dma_start(out_ext[:], res[:])

    nc.finalize()
    return nc


_NC_CACHE = {}


def _get_nc():
    if "nc" not in _NC_CACHE:
        _NC_CACHE["nc"] = _build()
    return _NC_CACHE["nc"]


def kernel(**inputs):
    inp = {k: np.ascontiguousarray(np.asarray(v, dtype=np.float32))
           for k, v in inputs.items()}
    nc = _get_nc()

    def bf(x):
        return np.ascontiguousarray(x.astype(NPBF))

    def f8(x):
        return np.ascontiguousarray(x.astype(NPF8))

    rep = {}
    for k in ["fc0_w", "fc1_w", "fc2_w", "agg0_w", "agg1_w"]:
        rep[k] = bf(inp[k])
    rep["gcnw2_8"] = f8(inp["gcn_w2"])

    cb = np.zeros((P, CB), np.float32)
    cb[:, 0:P] = np.eye(P)
    cb[:, 128:384] = inp["gcn_w1"].reshape(HK, P, E).transpose(
        1, 0, 2).reshape(P, HK * E)
    cb[0:E, 384:448] = inp["gcn_w2"]
    cb[0:E, 448:512] = inp["att_w"]
    cb[0:E, 512:576] = inp["proj_w"]
    cb[0:E, 576:592] = inp["mlp1_w"]
    cb[:, 592:1104] = inp["fc1_b"].reshape(1, H)
    cb[:, 1104:1616] = inp["fc2_b"].reshape(1, H)
    rep["cbf"] = bf(cb)

    # full transposed adjacencies (replicated, fp8)
    rep["adjF0"] = f8(inp["adj0"].T)
    rep["adjF1"] = f8(inp["madj0"].T)
    rep["adjF2"] = f8(inp["adj1"].T)
    rep["adjF3"] = f8(inp["madj1"].T)

    rec = np.stack(
        [1.0 / np.maximum(inp["nei0"].sum(1), 1.0),
         1.0 / np.maximum(inp["nei1"].sum(1), 1.0)], axis=1
    ).astype(np.float32)                                  # [N, 2]

    in_maps = []
    for r in range(C):
        rs = slice(r * NL, (r + 1) * NL)
        ms = slice(r * ML, (r + 1) * ML)
        d = dict(rep)
        cf = np.zeros((P, CW), np.float32)
        cf[:, 0:4] = inp["fc0_b"].reshape(HK, P).T
        cf[:, 4] = 0.5 * float(inp["mlp2_b"].reshape(-1)[0])
        cf[:, 5:21] = inp["mlp1_b"].reshape(1, 16)
        cf[:, 21:23] = rec[rs]
        cf[0:E, 23] = inp["gcn_b1"].reshape(E)
        cf[0:E, 24] = inp["gcn_b2"].reshape(E)
        cf[0:E, 25] = inp["att_b"].reshape(E)
        cf[0:E, 26] = inp["att_vec"].reshape(E)
        cf[0:E, 27] = inp["proj_b"].reshape(E)
        cf[0:E, 28] = 2.0 * inp["proj_b"].reshape(E)
        cf[:, 29:45] = inp["mlp2_w"].reshape(1, 16)
        d["cf32"] = np.ascontiguousarray(cf)
        d["fm0T"] = bf(np.concatenate(
            [inp["feat0"][rs].T, inp["mask_feat"][rs].T], axis=1))
        d["feat1T"] = bf(inp["feat1"][ms].T)
        d["feat2T"] = bf(inp["feat2"][ms].T)
        d["nei0T"] = f8(inp["nei0"][:, ms].T)
        d["nei1T"] = f8(inp["nei1"][:, ms].T)
        d["adjS0"] = f8(inp["adj0"][rs].T)
        d["adjS1"] = f8(inp["madj0"][rs].T)
        d["adjS2"] = f8(inp["adj1"][rs].T)
        d["adjS3"] = f8(inp["madj1"][rs].T)
        in_maps.append(d)

    trace = bool(int(os.environ.get("KERNEL_TRACE", "0")))
    res = run_bass_kernel_spmd(
        nc, in_maps, core_ids=list(range(C)), trace=trace)
    if trace:
        _NC_CACHE["exec_time_ns"] = res.exec_time_ns
        _NC_CACHE["trace"] = res.instructions_and_trace
    total = sum(float(res.results[r]["out"][0, 0]) for r in range(C))
    return np.float32(total / N)
